# revision 2
# baseline (speedup 1.0000x reference)
"""Multi-head attention (B=4, S=2048, E=768, H=12, D=64, causal) on 8 trn2
NeuronCores.

Sharding: core c -> batch b = c//2, head-half g = c%2 (6 heads each).
Each core computes its 6 heads' attention plus the partial output
projection; the host sums the two half-head partials per batch.

On-device strategy (per core), v2 (fp8-DoubleRow):
  - QKV projections run in residual-fp8 (x = xh+xl, W = wh+wl, keep the
    hh/lh/hl products, drop ll ~ 0.4%): e-chunk PAIRS feed fp8e4
    DoubleRow matmuls at 0.5 cyc/row -> 25% fewer PE cycles than f32r,
    and x ships at 2 B/elem instead of 4.
  - Scores also run residual-fp8 via a stacked DoubleRow trick:
      group 0 = [kh | kl^]^T [qh | qh^]  -> qh.kh + qh.kl
      group 1 = [kh | 0  ]^T [ql | 0  ]  -> ql.kh
    (top half = head's native partition range, ^ = DMA-shifted copy)
    one 256-cycle DR instr per 128k x 512q tile vs 512 f32r cycles.
    q/k are split to fp8 hi/lo on DVE straight out of the projection
    PSUM; the partition-shifted halves of the DR operand tiles are
    built with SBUF->SBUF DMAs on the ACT queue.
  - exp'd scores E stay f32r (fp8 E fails the tolerance: softmax spans
    e^-5..e^5 and fp8 subnormals destroy small weights), so the ctx
    matmuls (V_aug^T E^T, K=65) and out-proj stay f32r too.
  - The attention phase is ACT(exp)-bound, so the V projection is
    interleaved into the attention stream per q-window: exp starts
    right after the QK projection instead of after ALL projections.
  - A ones-column packed into V_aug yields softmax row-sums as ctx row
    64 for free; normalization: copy ctx'+rowsum out of PSUM early
    (frees the accumulator), reciprocal on DVE, partition-broadcast
    via a stride-0 DMA on the ACT queue, one DVE mul into ctxT.
  - Causal masking: fully-masked tiles skipped; diagonal tiles get a
    -1e9 strict-lower-triangle added via a bf16 matmul into the same
    PSUM accumulation group.
  - ctx matmuls are software-pipelined one step behind the scores
    matmuls; inputs ship hi/lo-combined, x on the SP queue and weights
    on the ACT queue, in first-use order.
  - One PSUM pool with fixed tags (16 KB/partition exactly) is shared
    by all phases so work pipelines through buffer rotation.
"""
import sys, json, os

for _p in ("/opt/trn_rl_repo",):
    if _p not in sys.path and os.path.isdir(_p):
        sys.path.insert(0, _p)

import numpy as np
import concourse.bass as bass
import concourse.mybir as mybir
import concourse.tile as tile
from concourse.bass_utils import run_bass_kernel_spmd

B, S, E, H, D = 4, 2048, 768, 12, 64
HPC = H // 2          # heads per core = 6
FPC = HPC * D         # features per core per q/k/v = 384
EC = E // 128         # 6 contraction chunks for projections
SC = S // 128         # 16 s-chunks
QW = S // 512         # 4 q-windows
KC = S // 128         # 16 k-chunks
F32 = mybir.dt.float32
F32R = mybir.dt.float32r
BF16 = mybir.dt.bfloat16
F8 = mybir.dt.float8e4
DRM = mybir.MatmulPerfMode.DoubleRow
AL = mybir.AluOpType
EXP = mybir.ActivationFunctionType.Exp
NEG = -1.0e9


def round_f32r(a: np.ndarray) -> np.ndarray:
    """Round fp32 -> fp32r (8 explicit mantissa bits), RNE, as fp32 bits."""
    a = np.ascontiguousarray(a, dtype=np.float32)
    u = a.view(np.uint32).astype(np.uint64)
    u2 = (u + 0x3FFF + ((u >> 15) & 1)) & np.uint64(0xFFFF8000)
    return u2.astype(np.uint32).view(np.float32)


def _patch_multiwait(nc, max_waits=1):
    """This container's walrus rejects instructions with more than one sync
    wait. Split excess waits onto same-engine NOPs emitted immediately
    before the instruction (same-engine streams are order-preserving)."""
    raw = nc.to_json_bytes()
    m = json.loads(raw)
    for f in m["functions"]:
        for b in f["blocks"]:
            out = []
            for inst in b["instructions"]:
                si = inst.get("sync_info") or {}
                ws = si.get("on_wait") or []
                if len(ws) > max_waits:
                    eng = inst["engine"]
                    for i, w in enumerate(ws[:-max_waits]):
                        out.append({
                            "debug": inst.get("debug", 0), "engine": eng,
                            "ins": [], "name": inst["name"] + f"-mw{i}",
                            "opcode": "NoOp", "outs": [],
                            "sync_info": {"on_update": [], "on_wait": [w]},
                        })
                    si["on_wait"] = ws[-max_waits:]
                out.append(inst)
            b["instructions"] = out
    patched = json.dumps(m).encode()
    nc.to_json_bytes = lambda: patched
    return nc


def build_nc(repeat=1, with_bias=True):
    nc = bass.Bass()
    x8 = nc.dram_tensor("x8", [128, EC, 2, S], F8, kind="ExternalInput")
    wqk8 = nc.dram_tensor("wqk8", [128, EC, 2, 2 * FPC], F8,
                          kind="ExternalInput")
    wv8 = nc.dram_tensor("wv8", [128, EC, 2, FPC], F8, kind="ExternalInput")
    woT = nc.dram_tensor("woT", [FPC, E], F32R, kind="ExternalInput")
    bqk = nc.dram_tensor("bqk", [128, 2 * FPC // 128], F32, kind="ExternalInput")
    bv = nc.dram_tensor("bv", [1, FPC], F32R, kind="ExternalInput")
    bo = nc.dram_tensor("bo", [1, E], F32R, kind="ExternalInput")
    tri = nc.dram_tensor("tri", [128, 128], BF16, kind="ExternalInput")
    ident = nc.dram_tensor("ident", [128, 128], BF16, kind="ExternalInput")
    ones = nc.dram_tensor("ones", [1, 128], F32R, kind="ExternalInput")
    y = nc.dram_tensor("y", [S, E], F32, kind="ExternalOutput")

    with tile.TileContext(nc) as tc, \
         nc.allow_low_precision(reason="fp8-residual matmul pipeline by design"):
        with tc.tile_pool(name="persist", bufs=1) as P, \
             tc.tile_pool(name="ps", bufs=1, space="PSUM") as PS:
            # --- persistent tiles (bottom-of-stack, live whole kernel)
            MD = [P.tile([128, 2, S], F8, name=f"MD{h}") for h in range(HPC)]
            SD = [P.tile([128, 2, S], F8, name=f"SD{h}") for h in range(HPC)]
            V_sb = [P.tile([128, 65 * HPC], F32R, name=f"V{i}") for i in range(KC)]
            ctxT_sb = [P.tile([128, S], F32R, name=f"ctxT{i}") for i in range(3)]
            woT_sb = [P.tile([128, E], F32R, name=f"woT{i}") for i in range(3)]
            bqk_sb = P.tile([128, 6], F32, name="bqk_sb")
            bv_sb = P.tile([1, FPC], F32R, name="bv_sb")
            bo_sb = P.tile([1, E], F32R, name="bo_sb")
            tri_sb = P.tile([128, 128], BF16, name="tri_sb")
            id_sb = P.tile([128, 128], BF16, name="id_sb")
            on_sb = P.tile([1, 128], F32R, name="on_sb")

            def ps_tile(shape, tag, bufs):
                return PS.tile(shape, F32, name=tag, tag=tag, bufs=bufs)

            # zero the never-written halves of the DR tiles (group 1's far
            # half must be 0.0 on BOTH operands so the PE never sees fp8
            # NaN garbage; group 0's far half gets real data via DMA).
            for h in range(HPC):
                R2 = 64 * (1 - (h % 2))   # far (non-native) partition range
                nc.gpsimd.memset(
                    MD[h][R2:R2 + 64, 1, :].bitcast(F32), 0.0)
                nc.gpsimd.memset(
                    SD[h][R2:R2 + 64, 1, :].bitcast(F32), 0.0)

            with tc.tile_pool(name="inpx", bufs=1) as PX:
                x8_sb = PX.tile([128, EC, 2, S], F8, name="x8_sb")
                wv8_sb = PX.tile([128, EC, 2, FPC], F8, name="wv8_sb")

                # ============== phase 1: qk projection ==============
                with tc.tile_pool(name="inpq", bufs=1) as PQ:
                    wqk8_sb = PQ.tile([128, EC, 2, 2 * FPC], F8,
                                      name="wqk8_sb")
                    # q/k fp8 hi/lo staging, one per fo chunk (0-2 q, 3-5 k)
                    st = [PQ.tile([128, 2, S], F8, name=f"st{i}")
                          for i in range(6)]

                    # x half-columns first (s 0:1024 feeds the first qk
                    # sub-chains AND v-proj sc 0..7), weights on the ACT
                    # queue; everything latency-tolerant rides SP.
                    for i in range(EC):
                        nc.sync.dma_start(x8_sb[:, i, :, 0:1024],
                                          x8.ap()[:, i, :, 0:1024])
                        nc.scalar.dma_start(wqk8_sb[:, i, :, :],
                                            wqk8.ap()[:, i, :, :])
                    for i in range(EC):
                        nc.sync.dma_start(x8_sb[:, i, :, 1024:S],
                                          x8.ap()[:, i, :, 1024:S])
                    nc.sync.dma_start(bqk_sb[:], bqk.ap())
                    for i in range(EC):
                        nc.scalar.dma_start(wv8_sb[:, i, :, :],
                                            wv8.ap()[:, i, :, :])
                    nc.sync.dma_start(bv_sb[:], bv.ap())
                    nc.sync.dma_start(tri_sb[:], tri.ap())
                    nc.sync.dma_start(id_sb[:], ident.ap())
                    nc.sync.dma_start(on_sb[:], ones.ap())
                    for i in range(3):
                        nc.sync.dma_start(woT_sb[i][:],
                                          woT.ap()[128 * i:128 * (i + 1), :])
                    nc.sync.dma_start(bo_sb[:], bo.ap())

                    def emit_qk(fo):
                        # two sequential half-S sub-chains of 9 DR instrs
                        # (3 e-pairs x 3 terms hh/lh/hl) x 2 windows: the
                        # second sub-chain runs while the first converts.
                        for hs in range(2):
                            pair = ps_tile([128, 1024], "pss_t", 2)
                            for p3 in range(3):
                                ee = slice(2 * p3, 2 * p3 + 2)
                                for t in range(3):
                                    wg, xg = ((0, 0), (0, 1), (1, 0))[t]
                                    for w2 in range(2):
                                        sw = 2 * hs + w2
                                        nc.tensor.matmul(
                                            pair[:, 512 * w2:512 * (w2 + 1)],
                                            wqk8_sb[:, ee, wg,
                                                    128 * fo:128 * (fo + 1)],
                                            x8_sb[:, ee, xg,
                                                  512 * sw:512 * (sw + 1)],
                                            start=(p3 == 0 and t == 0),
                                            stop=(p3 == 2 and t == 2),
                                            perf_mode=DRM,
                                            skip_group_check=True)
                            # fp8 hi/lo split: hi on ACT (idle until the
                            # first exp), lo residual on DVE
                            cs = slice(1024 * hs, 1024 * (hs + 1))
                            if with_bias:
                                nc.scalar.activation(
                                    st[fo][:, 0, cs], pair[:],
                                    mybir.ActivationFunctionType.Copy,
                                    bias=bqk_sb[:, fo:fo + 1])
                                nc.vector.scalar_tensor_tensor(
                                    st[fo][:, 1, cs], pair[:],
                                    bqk_sb[:, fo:fo + 1], st[fo][:, 0, cs],
                                    AL.add, AL.subtract)
                            else:
                                nc.scalar.copy(st[fo][:, 0, cs], pair[:])
                                nc.vector.scalar_tensor_tensor(
                                    st[fo][:, 1, cs], pair[:], 0.0,
                                    st[fo][:, 0, cs], AL.add, AL.subtract)

                    def emit_build(hp):
                        # assemble the score DR operand tiles for head pair
                        # hp from the q (fo=hp) and k (fo=3+hp) staging
                        fo = 3 + hp
                        for hd in range(2):
                            h = 2 * hp + hd
                            R, R2 = 64 * hd, 64 * (1 - hd)
                            rs = slice(R, R + 64)
                            rs2 = slice(R2, R2 + 64)
                            # SD g0 = [kh | kl^], g1 = [kh | 0]
                            nc.sync.dma_start(
                                SD[h][rs, :, :],
                                st[fo][rs, 0:1, :].broadcast_to([64, 2, S]))
                            nc.sync.dma_start(SD[h][rs2, 0, :],
                                              st[fo][rs, 1, :])
                            # MD g0 = [qh | qh^], g1 = [ql | 0]
                            nc.sync.dma_start(MD[h][rs, :, :],
                                              st[hp][rs, :, :])
                            nc.sync.dma_start(MD[h][rs2, 0, :],
                                              st[hp][rs, 0, :])

                    # phase 2 pools open while inpq is still live: the qk
                    # chains, v-proj and attention interleave so the ACT
                    # exp stream starts right after the first two head
                    # pairs' projections instead of after ALL projections.
                    with tc.tile_pool(name="esb", bufs=6) as EP, \
                         tc.tile_pool(name="nrm", bufs=4) as NP, \
                         tc.tile_pool(name="osb", bufs=3) as OP:

                    def emit_vproj(scp):
                        psvs = [ps_tile([128, FPC], "psc_t", 2)
                                for _ in range(2)]
                        for p3 in range(3):
                            ee = slice(2 * p3, 2 * p3 + 2)
                            for t in range(3):
                                xg, wg = ((0, 0), (1, 0), (0, 1))[t]
                                for p in range(2):
                                    sc = 2 * scp + p
                                    nc.tensor.matmul(
                                        psvs[p][:],
                                        x8_sb[:, ee, xg,
                                              128 * sc:128 * (sc + 1)],
                                        wv8_sb[:, ee, wg, :],
                                        start=(p3 == 0 and t == 0),
                                        stop=(not with_bias and p3 == 2
                                              and t == 2),
                                        perf_mode=DRM,
                                        skip_group_check=True)
                        for p in range(2):
                            sc = 2 * scp + p
                            if with_bias:
                                nc.tensor.matmul(psvs[p][:], on_sb[:, 0:128],
                                                 bv_sb[:], start=False,
                                                 stop=True,
                                                 skip_group_check=True)
                            vv = V_sb[sc][:].rearrange("p (h x) -> p h x",
                                                       x=65)
                            nc.vector.tensor_copy(
                                vv[:, :, 0:64],
                                psvs[p][:].rearrange("p (h x) -> p h x",
                                                     x=64))
                            nc.gpsimd.memset(vv[:, :, 64:65].bitcast(F32), 1.0)

                    def emit_scores(hp, qw, u):
                        """Scores (pair of k-chunks) for both heads + exp on
                        the [128,1024] pair tile. Returns {hd: (pss, E)}."""
                        Es = {}
                        for hd in range(2):
                            Es[hd] = (ps_tile([128, 1024], "pss_t", 2),
                                      EP.tile([128, 1024], F32R, name="E_t"))
                        for half in range(2):
                            ki = 2 * u + half
                            j = ki - 4 * qw
                            diag = j >= 0
                            for hd in range(2):
                                h = 2 * hp + hd
                                pss = Es[hd][0]
                                nc.tensor.matmul(
                                    pss[:, 512 * half:512 * (half + 1)],
                                    SD[h][:, :, 128 * ki:128 * (ki + 1)],
                                    MD[h][:, :, 512 * qw:512 * (qw + 1)],
                                    start=True, stop=not diag,
                                    perf_mode=DRM,
                                    skip_group_check=True)
                            if diag:
                                for hd in range(2):
                                    pss = Es[hd][0]
                                    nc.tensor.matmul(
                                        pss[:, 512 * half + 128 * j:
                                            512 * half + 128 * (j + 1)],
                                        id_sb[:], tri_sb[:],
                                        start=False, stop=True,
                                        skip_group_check=True)
                        for hd in range(2):
                            pss, Et = Es[hd]
                            j0 = 2 * u - 4 * qw
                            c0 = 128 * j0 if j0 > 0 else 0
                            # one exp spans both halves; the gap cols hold
                            # raw scores that no ctx matmul ever streams
                            nc.scalar.activation(Et[:, c0:1024],
                                                 pss[:, c0:1024],
                                                 EXP, scale=0.125)
                        return Es

                    def emit_ctx(hp, qw, u, Es, psc):
                        nki = 4 * qw + 4
                        for half in range(2):
                            ki = 2 * u + half
                            j = ki - 4 * qw
                            c = 128 * j if j > 0 else 0
                            for hd in range(2):
                                _, Et = Es[hd]
                                h = 2 * hp + hd
                                nc.tensor.matmul(
                                    psc[hd][:, c:512],
                                    V_sb[ki][:, 65 * h:65 * h + 65],
                                    Et[:, 512 * half + c:512 * (half + 1)],
                                    start=(ki == 0), stop=(ki == nki - 1),
                                    skip_group_check=True)

                    def emit_norm(hp, qw, psc):
                        """Copy ctx'+rowsum out of PSUM (freeing it), then
                        reciprocal -> stride-0 DMA broadcast -> normalize."""
                        for hd in range(2):
                            craw = NP.tile([65, 512], F32, name="craw_t")
                            nc.vector.tensor_copy(craw[:], psc[hd][:])
                            rinv = NP.tile([1, 512], F32R, name="rinv_t")
                            nc.vector.reciprocal(rinv[:], craw[64:65, :])
                            bc = NP.tile([64, 512], F32R, name="bc_t")
                            nc.scalar.dma_start(
                                bc[:],
                                rinv[:, None, :].broadcast_to([1, 64, 512]))
                            nc.vector.tensor_mul(
                                ctxT_sb[hp][64 * hd:64 * (hd + 1),
                                            512 * qw:512 * (qw + 1)],
                                craw[0:64, :], bc[:])

                    def emit_outproj(qw):
                        for sc in range(4 * qw, 4 * qw + 4):
                            osb = OP.tile([128, E], F32, name="osb_t")
                            pos = {0: ps_tile([128, 512], "po_t", 2),
                                   512: ps_tile([128, 512], "po_t", 2)}
                            for c in range(3):
                                for f0, fn in ((0, 512), (512, 256)):
                                    nc.tensor.matmul(
                                        pos[f0][:, 0:fn],
                                        ctxT_sb[c][:, 128 * sc:128 * (sc + 1)],
                                        woT_sb[c][:, f0:f0 + fn],
                                        start=(c == 0),
                                        stop=(not with_bias and c == 2),
                                        skip_group_check=True)
                            for f0, fn in ((0, 512), (512, 256)):
                                if with_bias:
                                    nc.tensor.matmul(pos[f0][:, 0:fn],
                                                     on_sb[:, 0:128],
                                                     bo_sb[:, f0:f0 + fn],
                                                     start=False, stop=True,
                                                     skip_group_check=True)
                                nc.vector.tensor_copy(osb[:, f0:f0 + fn],
                                                      pos[f0][:, 0:fn])
                            nc.sync.dma_start(
                                y.ap()[128 * sc:128 * (sc + 1), :], osb[:])

                    def emit_attention():
                        # software pipeline: ctx trails scores by one step
                        DEPTH = 1
                        pending = []   # [(hp, qw, u, Es, psc, last_u), ...]

                        def flush_one():
                            php, pqw, pu, pEs, ppsc, plast = pending.pop(0)
                            emit_ctx(php, pqw, pu, pEs, ppsc)
                            if pu == plast:
                                emit_norm(php, pqw, ppsc)
                                if php == 2:
                                    emit_outproj(pqw)

                        for qw in range(QW):
                            emit_vproj(2 * qw)
                            emit_vproj(2 * qw + 1)
                            for hp in range(3):
                                nu = (4 * qw + 4) // 2
                                psc = {hd: ps_tile([65, 512], "psc_t", 2)
                                       for hd in range(2)}
                                for u in range(nu):
                                    Es = emit_scores(hp, qw, u)
                                    if len(pending) >= DEPTH:
                                        flush_one()
                                    pending.append(
                                        (hp, qw, u, Es, psc, nu - 1))
                        while pending:
                            flush_one()

                    if repeat == 1:
                        emit_attention()
                    else:
                        with tc.For_i(0, repeat, 1):
                            emit_attention()

    return _patch_multiwait(nc)


_NC = {}


def _get_nc(with_bias=True):
    if with_bias not in _NC:
        _NC[with_bias] = build_nc(with_bias=with_bias)
    return _NC[with_bias]


def _split8_pack(a2d):
    """[E_rows, C] f32 -> [128, EC_rows, 2, C] uint8: per-e-chunk partition-
    major packing of the fp8e4 residual split (hi at [:,:,0], lo [:,:,1])."""
    import ml_dtypes
    f8 = ml_dtypes.float8_e4m3
    a = np.ascontiguousarray(a2d, dtype=np.float32)
    hi = np.clip(a, -240, 240).astype(f8)
    lo = np.clip(a - hi.astype(np.float32), -240, 240).astype(f8)
    r, c = a.shape
    n = r // 128
    out = np.empty((128, n, 2, c), np.uint8)
    out[:, :, 0, :] = hi.view(np.uint8).reshape(n, 128, c).transpose(1, 0, 2)
    out[:, :, 1, :] = lo.view(np.uint8).reshape(n, 128, c).transpose(1, 0, 2)
    return out


def _prep_core_inputs(x, in_proj_w, in_proj_b, out_w, out_b):
    """Build the 8 per-core input dicts (host-side shard + split + pack)."""
    import ml_dtypes
    tri_np = np.where(np.arange(128)[None, :] < np.arange(128)[:, None],
                      np.float32(NEG), np.float32(0.0))
    tri_bf = tri_np.astype(ml_dtypes.bfloat16)
    id_bf = np.eye(128, dtype=np.float32).astype(ml_dtypes.bfloat16)
    ones_np = round_f32r(np.ones((1, 128), np.float32))

    x8_by_b = [_split8_pack(np.asarray(x[b]).T) for b in range(B)]

    in_maps = []
    for c in range(8):
        b = c // 2
        g = c % 2
        f0 = FPC * g
        Wq = np.asarray(in_proj_w[f0:f0 + FPC])
        Wk = np.asarray(in_proj_w[E + f0:E + f0 + FPC])
        Wv = np.asarray(in_proj_w[2 * E + f0:2 * E + f0 + FPC])
        bq = np.asarray(in_proj_b[f0:f0 + FPC])
        bk = np.asarray(in_proj_b[E + f0:E + f0 + FPC])
        bvv = np.asarray(in_proj_b[2 * E + f0:2 * E + f0 + FPC])
        Wo = np.asarray(out_w[:, f0:f0 + FPC])
        bqk_np = np.concatenate([bq, bk]).astype(np.float32).reshape(6, 128).T
        in_maps.append({
            "x8": x8_by_b[b],
            "wqk8": _split8_pack(np.concatenate([Wq, Wk], axis=0).T),
            "wv8": _split8_pack(Wv.T),
            "woT": round_f32r(Wo.T),
            "bqk": np.ascontiguousarray(bqk_np),
            "bv": round_f32r(bvv.reshape(1, FPC)),
            # out bias only on even cores so the host-side pair-sum is exact
            "bo": round_f32r(np.asarray(out_b).reshape(1, E)) if g == 0
                  else np.zeros((1, E), np.float32),
            "tri": tri_bf,
            "ident": id_bf,
            "ones": ones_np,
        })
    return in_maps


def kernel(x, in_proj_w, in_proj_b, out_w, out_b):
    zero_bias = (not np.any(np.asarray(in_proj_b))) and \
                (not np.any(np.asarray(out_b)))
    nc = _get_nc(with_bias=not zero_bias)
    in_maps = _prep_core_inputs(x, in_proj_w, in_proj_b, out_w, out_b)
    res = run_bass_kernel_spmd(nc, in_maps, core_ids=list(range(8)))
    out = np.empty((B, S, E), np.float32)
    for b in range(B):
        out[b] = res.results[2 * b]["y"] + res.results[2 * b + 1]["y"]
    return out
